# revision 12
# baseline (speedup 1.0000x reference)
"""GRU unit kernel for Trainium2, data-parallel over 8 NeuronCores.

Computation (per batch row):
    r  = sigmoid(x @ W_i2r + b_i2r + h @ W_h2r)
    z  = sigmoid(x @ W_i2z + b_i2z + h @ W_h2z)
    h1 = tanh   (x @ W_i2h + b_i2h + r * (h @ W_h2h))
    out = (1 - z) * h1 + z * h

Sharding: batch (16384) split 8 ways; weights replicated.

The contraction (K=1024 per side) is split bf16 + fp8: the first KB{X,H}
rows run as bf16 matmuls, the last KF{X,H} rows as fp8e4 DoubleRow matmuls
(2 fp8 MACs/cell/cycle -> a K=256 DR matmul costs the same ~270ns as one
K=128 bf16 matmul; measured). fp8 operands are pre-scaled x/4, w*4 on the
host so products come out unscaled and accumulate into the same PSUM bank;
e4m3 on ~N(0,1)/4 activations and U(+-1/8) weights stays in the normal
range. Measured end-to-end rel err 1.7286e-2 (KFX=512, KFH=256) vs the
2e-2 gate - identical on HW and in numpy emulation (deterministic), with
(256,256) at 1.40e-2 as a fallback. HW exec: 343-346us vs 438.7us for the
all-bf16 version (the bf16 matmul stream alone measures 414us).

Host-side prep: x/h transposed to [K, B_local] (so the stationary matmul
operand loads directly), weight matrices concatenated to [K, 3072], biases
pre-broadcast to [128, 3072] f32, h also shipped row-major bf16 for the
blend; output returned bf16 (converted to f32 on host).

Device kernel per core (B_local=2048 rows = 16 m-tiles of 128):
  - weights + xT/hT resident in SBUF, h-blend (bf16) and out streamed.
  - per m-tile: accumulate pre_r, pre_z, x@W_i2h, r-side h@W_h2h into 8
    PSUM banks (each gate split in two 512-col halves); epilogue on DVE
    (bias adds, blend) + ACT (sigmoid/tanh); out DMA'd in bf16.
  - LDWEIGHTS are fully hidden by the PE's background weight buffer
    (measured: 1536 vs 511 LDWs for the same matmuls = identical time),
    so stationary-sharing order is chosen purely for DMA-arrival overlap.
"""

import os
import numpy as np
import ml_dtypes
from contextlib import ExitStack

import concourse.bass as bass
import concourse.tile as tile
from concourse import bacc, mybir

N_CORES = 8
B, I, H = 16384, 1024, 1024
BL = B // N_CORES           # 2048 batch rows per core
MT = BL // 128              # 16 m-tiles
KO = I // 128               # 8 k-tiles of 128 (total contraction)

# fp8 DoubleRow split: last KFX (x side) / KFH (h side) contraction rows
# run in fp8e4; must be multiples of 256 (a DR matmul consumes a k-pair).
KFX = int(os.environ.get("GRU_KFX", "512"))
KFH = int(os.environ.get("GRU_KFH", "256"))
S8 = 4.0                    # host pre-scale: x/S8, w*S8
KBX, KBH = I - KFX, H - KFH
KOX, KOH = KBX // 128, KBH // 128
KPX, KPH = KFX // 256, KFH // 256

F32 = mybir.dt.float32
BF16 = mybir.dt.bfloat16
FP8 = mybir.dt.float8e4
BF16_NP = ml_dtypes.bfloat16
FP8_NP = ml_dtypes.float8_e4m3fn
DRMODE = mybir.MatmulPerfMode.DoubleRow


def _ap_key(a):
    try:
        return (a.memref, a.offset, str(a.ap), str(a.dtype))
    except Exception:
        return ("?", id(a))


def dedupe_ldweights(nc):
    """Drop InstLdweights that reload the stationary tile already resident in
    the PE array (bacc emits one per matmul). The paired InstMatmult keeps
    both APs, so data deps survive; the removed LDW's scheduling deps are
    merged into the following instruction. (LDWs are free on HW anyway -
    this mainly trims instruction-fetch volume.)"""
    total_removed = 0
    for blk in nc.m.functions[0].blocks:
        insts = list(blk.instructions)
        new = []
        last_key = None
        pending = []
        for i in insts:
            t = type(i).__name__
            eng = str(getattr(i, "engine", ""))
            if t == "InstLdweights":
                key = (_ap_key(i.ins[0]), str(i.perf_mode),
                       str(i.tile_position), str(i.is_transpose))
                if key == last_key:
                    pending.append(i)
                    total_removed += 1
                    continue
                last_key = key
                new.append(i)
            else:
                if "PE" in eng and t not in ("InstMatmult",
                                             "InstEventSemaphore"):
                    last_key = None  # unknown PE inst may clobber weights
                if pending and t == "InstMatmult":
                    for j in pending:
                        i.merge_dependencies_from(j)
                    pending = []
                new.append(i)
        if pending:
            new.extend(pending)
        blk.instructions = new
    return total_removed


def build_nc(reps: int = 1):
    nc = bacc.Bacc("TRN2", target_bir_lowering=False, debug=False,
                   num_devices=N_CORES)
    AF = mybir.ActivationFunctionType

    xt = nc.dram_tensor("xt", [KBX, BL], BF16, kind="ExternalInput").ap()
    ht = nc.dram_tensor("ht", [KBH, BL], BF16, kind="ExternalInput").ap()
    hb = nc.dram_tensor("hb", [BL, H], BF16, kind="ExternalInput").ap()
    wx = nc.dram_tensor("wx", [KBX, 3 * H], BF16, kind="ExternalInput").ap()
    wh = nc.dram_tensor("wh", [KBH, 3 * H], BF16, kind="ExternalInput").ap()
    bias = nc.dram_tensor("bias", [128, 3 * H], F32, kind="ExternalInput").ap()
    out = nc.dram_tensor("out", [BL, H], BF16, kind="ExternalOutput").ap()
    if KFX:
        xt8 = nc.dram_tensor("xt8", [KFX, BL], FP8, kind="ExternalInput").ap()
        wx8 = nc.dram_tensor("wx8", [KFX, 3 * H], FP8,
                             kind="ExternalInput").ap()
    if KFH:
        ht8 = nc.dram_tensor("ht8", [KFH, BL], FP8, kind="ExternalInput").ap()
        wh8 = nc.dram_tensor("wh8", [KFH, 3 * H], FP8,
                             kind="ExternalInput").ap()

    with tile.TileContext(nc) as tc, ExitStack() as ctx:
        wpool = ctx.enter_context(tc.tile_pool(name="w", bufs=1))
        apool = ctx.enter_context(tc.tile_pool(name="a", bufs=1))
        hpool = ctx.enter_context(tc.tile_pool(name="h", bufs=2))
        epool = ctx.enter_context(tc.tile_pool(name="e", bufs=3))
        psum = ctx.enter_context(tc.tile_pool(name="ps", bufs=1, space="PSUM"))

        wx_sb = wpool.tile([128, KOX, 3 * H], BF16, tag="wx")
        wh_sb = wpool.tile([128, KOH, 3 * H], BF16, tag="wh")
        bias_sb = wpool.tile([128, 3 * H], F32, tag="bias")
        xt_sb = apool.tile([128, KOX, BL], BF16, tag="xt")
        ht_sb = apool.tile([128, KOH, BL], BF16, tag="ht")
        if KFX:
            wx8_sb = wpool.tile([128, 2 * KPX, 3 * H], FP8, tag="wx8")
            xt8_sb = apool.tile([128, 2 * KPX, BL], FP8, tag="xt8")
        if KFH:
            wh8_sb = wpool.tile([128, 2 * KPH, 3 * H], FP8, tag="wh8")
            ht8_sb = apool.tile([128, 2 * KPH, BL], FP8, tag="ht8")

        xt_r = xt.rearrange("(ko ki) b -> ki ko b", ki=128)
        ht_r = ht.rearrange("(ko ki) b -> ki ko b", ki=128)
        wx_r = wx.rearrange("(ko ki) n -> ki ko n", ki=128)
        wh_r = wh.rearrange("(ko ki) n -> ki ko n", ki=128)
        if KFX:
            xt8_r = xt8.rearrange("(ko ki) b -> ki ko b", ki=128)
            wx8_r = wx8.rearrange("(ko ki) n -> ki ko n", ki=128)
        if KFH:
            ht8_r = ht8.rearrange("(ko ki) b -> ki ko b", ki=128)
            wh8_r = wh8.rearrange("(ko ki) n -> ki ko n", ki=128)

        def emit_loads():
            # Load order sets DMA FIFO order on the sync ring. m-tile 0 runs
            # gate-major (r, z, a) over 512-col weight halves, so stream in
            # exactly that consumption order; fine chunks let the next rep's
            # first m-tiles start while the rest still loads.
            CH = 4
            cw = BL // CH
            nc.sync.dma_start(xt_sb[:, :, 0:cw], xt_r[:, :, 0:cw])
            if KFX:
                nc.sync.dma_start(xt8_sb[:, :, 0:cw], xt8_r[:, :, 0:cw])
            for g in range(3):
                for nh in range(2):
                    c0 = g * H + nh * 512
                    nc.sync.dma_start(wx_sb[:, :, c0:c0 + 512],
                                      wx_r[:, :, c0:c0 + 512])
                    if KFX:
                        nc.sync.dma_start(wx8_sb[:, :, c0:c0 + 512],
                                          wx8_r[:, :, c0:c0 + 512])
            nc.sync.dma_start(ht_sb[:, :, 0:cw], ht_r[:, :, 0:cw])
            if KFH:
                nc.sync.dma_start(ht8_sb[:, :, 0:cw], ht8_r[:, :, 0:cw])
            for g in range(3):
                for nh in range(2):
                    c0 = g * H + nh * 512
                    nc.sync.dma_start(wh_sb[:, :, c0:c0 + 512],
                                      wh_r[:, :, c0:c0 + 512])
                    if KFH:
                        nc.sync.dma_start(wh8_sb[:, :, c0:c0 + 512],
                                          wh8_r[:, :, c0:c0 + 512])
            nc.sync.dma_start(bias_sb[:], bias)
            for c in range(1, CH):
                cs = slice(c * cw, (c + 1) * cw)
                nc.sync.dma_start(xt_sb[:, :, cs], xt_r[:, :, cs])
                nc.sync.dma_start(ht_sb[:, :, cs], ht_r[:, :, cs])
                if KFX:
                    nc.sync.dma_start(xt8_sb[:, :, cs], xt8_r[:, :, cs])
                if KFH:
                    nc.sync.dma_start(ht8_sb[:, :, cs], ht8_r[:, :, cs])

        def emit_mtile(mt):
            ms = slice(mt * 128, (mt + 1) * 128)
            hb_t = hpool.tile([128, H], BF16, tag="hb")
            nc.sync.dma_start(hb_t[:], hb[ms, :])

            # 8 PSUM banks: r/z accumulate x-side + h-side; 'a' is x@W_i2h
            # (+bias later), 'b' is h@W_h2h (scaled by r later). Each bank
            # takes KOX+KPX x-side ops and/or KOH+KPH h-side ops.
            ps = {}
            for g in ("r", "z", "a", "b"):
                for nh in range(2):
                    ps[(g, nh)] = psum.tile([128, 512], F32, tag=f"p{g}{nh}",
                                            name=f"p{g}{nh}")

            def mm_x(gi, g, ko, nh):
                o = gi * H + nh * 512
                nc.tensor.matmul(ps[(g, nh)], xt_sb[:, ko, ms],
                                 wx_sb[:, ko, o:o + 512],
                                 start=(ko == 0),
                                 stop=(g == "a" and KPX == 0
                                       and ko == KOX - 1))

            def mm_x8(gi, g, kp, nh):
                o = gi * H + nh * 512
                nc.tensor.matmul(ps[(g, nh)], xt8_sb[:, 2 * kp:2 * kp + 2, ms],
                                 wx8_sb[:, 2 * kp:2 * kp + 2, o:o + 512],
                                 start=False,
                                 stop=(g == "a" and kp == KPX - 1),
                                 perf_mode=DRMODE)

            def mm_h(gi, g, ko, nh):
                o = gi * H + nh * 512
                nc.tensor.matmul(ps[(g, nh)], ht_sb[:, ko, ms],
                                 wh_sb[:, ko, o:o + 512],
                                 start=(g == "b" and ko == 0),
                                 stop=(KPH == 0 and ko == KOH - 1))

            def mm_h8(gi, g, kp, nh):
                o = gi * H + nh * 512
                nc.tensor.matmul(ps[(g, nh)], ht8_sb[:, 2 * kp:2 * kp + 2, ms],
                                 wh8_sb[:, 2 * kp:2 * kp + 2, o:o + 512],
                                 start=False,
                                 stop=(kp == KPH - 1),
                                 perf_mode=DRMODE)

            def x_side_gate_major():
                for gi, g in enumerate(("r", "z", "a")):
                    for nh in range(2):
                        for ko in range(KOX):
                            mm_x(gi, g, ko, nh)
                        for kp in range(KPX):
                            mm_x8(gi, g, kp, nh)

            def x_side_ko_major():
                for ko in range(KOX):
                    for nh in range(2):
                        for gi, g in enumerate(("r", "z", "a")):
                            mm_x(gi, g, ko, nh)
                for kp in range(KPX):
                    for nh in range(2):
                        for gi, g in enumerate(("r", "z", "a")):
                            mm_x8(gi, g, kp, nh)

            def h_side_ko_major():
                for ko in range(KOH):
                    for nh in range(2):
                        for gi, g in enumerate(("r", "z", "b")):
                            mm_h(gi, g, ko, nh)
                for kp in range(KPH):
                    for nh in range(2):
                        for gi, g in enumerate(("r", "z", "b")):
                            mm_h8(gi, g, kp, nh)

            def h_side_gate_major(order=("r", "z", "b")):
                h_idx = {"r": 0, "z": 1, "b": 2}
                for g in order:
                    for nh in range(2):
                        for ko in range(KOH):
                            mm_h(h_idx[g], g, ko, nh)
                        for kp in range(KPH):
                            mm_h8(h_idx[g], g, kp, nh)

            if mt == 0:
                # Gate-major: PE starts after ~1MB of weights instead of
                # all of them (extra LDWEIGHTS are free - fully hidden).
                x_side_gate_major()
                h_side_gate_major()
            else:
                x_side_ko_major()
                if mt < MT - 1:
                    h_side_ko_major()
                else:
                    # Last m-tile: finish 'b' first so the epilogue's
                    # critical chain (r*b -> tanh -> blend) starts sooner.
                    h_side_gate_major(order=("b", "r", "z"))

            for nh in range(2):
                o = nh * 512
                nsl = slice(o, o + 512)
                pr, pz = ps[("r", nh)], ps[("z", nh)]
                pa, pb = ps[("a", nh)], ps[("b", nh)]
                tr = epool.tile([128, 512], F32, tag="tr")
                tz = epool.tile([128, 512], F32, tag="tz")
                ta = epool.tile([128, 512], F32, tag="ta")
                nc.vector.tensor_add(tr[:], pr[:], bias_sb[:, o:o + 512])
                nc.scalar.activation(tr[:], tr[:], AF.Sigmoid)       # r
                nc.vector.tensor_add(tz[:], pz[:], bias_sb[:, H + o:H + o + 512])
                nc.scalar.activation(tz[:], tz[:], AF.Sigmoid)       # z
                nc.vector.tensor_add(ta[:], pa[:],
                                     bias_sb[:, 2 * H + o:2 * H + o + 512])
                nc.vector.tensor_mul(tr[:], tr[:], pb[:])            # r*(hU)
                nc.vector.tensor_add(ta[:], ta[:], tr[:])
                nc.scalar.activation(ta[:], ta[:], AF.Tanh)          # h1
                nc.vector.tensor_sub(tr[:], hb_t[:, nsl], ta[:])     # h-h1
                nc.vector.tensor_mul(tr[:], tz[:], tr[:])            # z*(h-h1)
                to = epool.tile([128, 512], BF16, tag="to")
                nc.vector.tensor_add(to[:], ta[:], tr[:])            # out
                nc.scalar.dma_start(out[ms, nsl], to[:])

        def body():
            emit_loads()
            for mt in range(MT):
                emit_mtile(mt)

        if reps > 1:
            # tc.For_i has an all-engine barrier per iteration, which blocks
            # the next rep's input DMA from overlapping this rep's tail.
            # Unroll several bodies per iteration so tile-level region deps
            # pipeline rep boundaries for (unroll-1)/unroll of the reps.
            unroll = 1
            pref = int(os.environ.get("GRU_UNROLL", "4"))
            for u in (pref, 4, 3, 2):
                if u > 1 and reps % u == 0:
                    unroll = u
                    break
            with tc.For_i(0, reps // unroll, 1):
                for _ in range(unroll):
                    body()
        else:
            body()

    nc.compile()
    if os.environ.get("GRU_DEDUP", "1") == "1":
        dedupe_ldweights(nc)
    return nc


def prep_in_maps(inputs):
    """Host-side marshalling: shard batch, transpose+cast activations,
    concat weights/biases, split the fp8 contraction rows. Returns per-core
    input dicts."""
    g = {k: np.asarray(v) for k, v in inputs.items()}
    x, h = g["inputs"], g["hidden"]
    wx = np.ascontiguousarray(np.concatenate(
        [g["W_i2r"], g["W_i2z"], g["W_i2h"]], axis=1)).astype(np.float32)
    wh = np.ascontiguousarray(np.concatenate(
        [g["W_h2r"], g["W_h2z"], g["W_h2h"]], axis=1)).astype(np.float32)
    b = np.concatenate([g["b_i2r"], g["b_i2z"], g["b_i2h"]]).astype(np.float32)
    bias_b = np.ascontiguousarray(np.broadcast_to(b, (128, 3 * H)))
    xt_all = np.ascontiguousarray(x.T).astype(np.float32)
    ht_all = np.ascontiguousarray(h.T).astype(np.float32)

    shared = {
        "wx": wx[:KBX].astype(BF16_NP),
        "wh": wh[:KBH].astype(BF16_NP),
        "bias": bias_b,
    }
    if KFX:
        shared["wx8"] = (wx[KBX:] * S8).astype(FP8_NP)
    if KFH:
        shared["wh8"] = (wh[KBH:] * S8).astype(FP8_NP)

    in_maps = []
    for c in range(N_CORES):
        sl = slice(c * BL, (c + 1) * BL)
        m = {
            "xt": np.ascontiguousarray(xt_all[:KBX, sl]).astype(BF16_NP),
            "ht": np.ascontiguousarray(ht_all[:KBH, sl]).astype(BF16_NP),
            "hb": np.ascontiguousarray(h[sl]).astype(BF16_NP),
        }
        if KFX:
            m["xt8"] = np.ascontiguousarray(
                xt_all[KBX:, sl] / S8).astype(FP8_NP)
        if KFH:
            m["ht8"] = np.ascontiguousarray(
                ht_all[KBH:, sl] / S8).astype(FP8_NP)
        m.update(shared)
        in_maps.append(m)
    return in_maps


_RUNNERS = {}


def get_runner(reps: int = 1):
    """Build the bass module once and wrap it in a jitted 8-way shard_map,
    mirroring concourse.bass2jax.run_bass_via_pjrt but reusable across calls
    (so repeated executions don't re-trace/re-compile). reps>1 wraps the
    whole kernel in an on-device loop (for timing via amortization)."""
    if reps in _RUNNERS:
        return _RUNNERS[reps]
    import jax
    from jax.sharding import Mesh, PartitionSpec
    from jax.experimental.shard_map import shard_map
    from concourse.bass2jax import (_bass_exec_p, install_neuronx_cc_hook,
                                    partition_id_tensor)

    nc = build_nc(reps)
    install_neuronx_cc_hook()

    partition_name = (nc.partition_id_tensor.name
                      if nc.partition_id_tensor else None)
    in_names, out_names, out_avals, zero_outs = [], [], [], []
    for alloc in nc.m.functions[0].allocations:
        if not isinstance(alloc, mybir.MemoryLocationSet):
            continue
        name = alloc.memorylocations[0].name
        if alloc.kind == "ExternalInput":
            if name != partition_name:
                in_names.append(name)
        elif alloc.kind == "ExternalOutput":
            out_names.append(name)
            shape = tuple(alloc.tensor_shape)
            dtype = mybir.dt.np(alloc.dtype)
            out_avals.append(jax.core.ShapedArray(shape, dtype))
            zero_outs.append(np.zeros(shape, dtype))
    all_names = in_names + out_names
    if partition_name is not None:
        all_names = all_names + [partition_name]
    all_names = tuple(all_names)
    n_in, n_out = len(in_names), len(out_names)

    def _body(*args):
        operands = list(args)
        if partition_name is not None:
            operands.append(partition_id_tensor())
        outs = _bass_exec_p.bind(
            *operands,
            out_avals=tuple(out_avals),
            in_names=all_names,
            out_names=tuple(out_names),
            lowering_input_output_aliases=(),
            sim_require_finite=True,
            sim_require_nnan=True,
            nc=nc,
        )
        return tuple(outs)

    devices = jax.devices()[:N_CORES]
    mesh = Mesh(np.asarray(devices), ("core",))
    sharded = jax.jit(
        shard_map(_body, mesh=mesh,
                  in_specs=(PartitionSpec("core"),) * (n_in + n_out),
                  out_specs=(PartitionSpec("core"),) * n_out,
                  check_rep=False),
        donate_argnums=tuple(range(n_in, n_in + n_out)),
        keep_unused=True,
    )
    _RUNNERS[reps] = (sharded, in_names, out_names, zero_outs)
    return _RUNNERS[reps]


def run_on_device(in_maps):
    sharded, in_names, out_names, zero_outs = get_runner()
    concat_in = [np.concatenate([m[n] for m in in_maps], axis=0)
                 for n in in_names]
    concat_zero = [np.zeros((N_CORES * z.shape[0], *z.shape[1:]), z.dtype)
                   for z in zero_outs]
    outs = sharded(*concat_in, *concat_zero)
    return {n: np.asarray(o) for n, o in zip(out_names, outs)}


_NC = None


def kernel(**inputs):
    """Full-input entry point: shard, run on 8 NeuronCores, gather."""
    global _NC
    from concourse._compat import axon_active
    in_maps = prep_in_maps(inputs)
    if axon_active():
        # PJRT path with a process-cached jitted executable (repeat calls
        # skip re-trace/re-compile).
        return run_on_device(in_maps)["out"].astype(np.float32)
    from concourse.bass_utils import run_bass_kernel_spmd
    if _NC is None:
        _NC = build_nc(1)
    res = run_bass_kernel_spmd(_NC, in_maps, core_ids=list(range(N_CORES)))
    return np.concatenate([res.results[c]["out"] for c in range(N_CORES)],
                          axis=0).astype(np.float32)


# revision 15
# speedup vs baseline: 1.0364x; 1.0364x over previous
"""GRU unit kernel for Trainium2, data-parallel over 8 NeuronCores.

Computation (per batch row):
    r  = sigmoid(x @ W_i2r + b_i2r + h @ W_h2r)
    z  = sigmoid(x @ W_i2z + b_i2z + h @ W_h2z)
    h1 = tanh   (x @ W_i2h + b_i2h + r * (h @ W_h2h))
    out = (1 - z) * h1 + z * h

Sharding: batch (16384) split 8 ways; weights replicated.

The contraction (K=1024 per side) is split bf16 + fp8: the first KB{X,H}
rows run as bf16 matmuls, the last KF{X,H} rows as fp8e4 DoubleRow matmuls
(2 fp8 MACs/cell/cycle -> a K=256 DR matmul costs the same ~270ns as one
K=128 bf16 matmul; measured). fp8 operands are pre-scaled x/4, w*4 on the
host so products come out unscaled and accumulate into the same PSUM bank;
e4m3 on ~N(0,1)/4 activations and U(+-1/8) weights stays in the normal
range. Measured end-to-end rel err 1.7286e-2 (KFX=512, KFH=256) vs the
2e-2 gate - identical on HW and in numpy emulation (deterministic), with
(256,256) at 1.40e-2 as a fallback. HW exec: 343-346us vs 438.7us for the
all-bf16 version (the bf16 matmul stream alone measures 414us).

Host-side prep: x/h transposed to [K, B_local] (so the stationary matmul
operand loads directly), weight matrices concatenated to [K, 3072], biases
pre-broadcast to [128, 3072] f32, h also shipped row-major bf16 for the
blend; output returned bf16 (converted to f32 on host).

Device kernel per core (B_local=2048 rows = 16 m-tiles of 128):
  - weights + xT/hT resident in SBUF, h-blend (bf16) and out streamed.
  - per m-tile: accumulate pre_r, pre_z, x@W_i2h, r-side h@W_h2h into 8
    PSUM banks (each gate split in two 512-col halves); epilogue on DVE
    (bias adds, blend) + ACT (sigmoid/tanh); out DMA'd in bf16.
  - LDWEIGHTS are fully hidden by the PE's background weight buffer
    (measured: 1536 vs 511 LDWs for the same matmuls = identical time),
    so stationary-sharing order is chosen purely for DMA-arrival overlap.
"""

import os
import numpy as np
import ml_dtypes
from contextlib import ExitStack

import concourse.bass as bass
import concourse.tile as tile
from concourse import bacc, mybir

N_CORES = 8
B, I, H = 16384, 1024, 1024
BL = B // N_CORES           # 2048 batch rows per core
MT = BL // 128              # 16 m-tiles
KO = I // 128               # 8 k-tiles of 128 (total contraction)

# fp8 DoubleRow split: last KFX (x side) / KFH (h side) contraction rows
# run in fp8e4; must be multiples of 256 (a DR matmul consumes a k-pair).
KFX = int(os.environ.get("GRU_KFX", "512"))
KFH = int(os.environ.get("GRU_KFH", "256"))
S8 = 4.0                    # host pre-scale: x/S8, w*S8
KBX, KBH = I - KFX, H - KFH
KOX, KOH = KBX // 128, KBH // 128
KPX, KPH = KFX // 256, KFH // 256

F32 = mybir.dt.float32
BF16 = mybir.dt.bfloat16
FP8 = mybir.dt.float8e4
BF16_NP = ml_dtypes.bfloat16
FP8_NP = ml_dtypes.float8_e4m3fn
DRMODE = mybir.MatmulPerfMode.DoubleRow


def _ap_key(a):
    try:
        return (a.memref, a.offset, str(a.ap), str(a.dtype))
    except Exception:
        return ("?", id(a))


def dedupe_ldweights(nc):
    """Drop InstLdweights that reload the stationary tile already resident in
    the PE array (bacc emits one per matmul). The paired InstMatmult keeps
    both APs, so data deps survive; the removed LDW's scheduling deps are
    merged into the following instruction. (LDWs are free on HW anyway -
    this mainly trims instruction-fetch volume.)"""
    total_removed = 0
    for blk in nc.m.functions[0].blocks:
        insts = list(blk.instructions)
        new = []
        last_key = None
        pending = []
        for i in insts:
            t = type(i).__name__
            eng = str(getattr(i, "engine", ""))
            if t == "InstLdweights":
                key = (_ap_key(i.ins[0]), str(i.perf_mode),
                       str(i.tile_position), str(i.is_transpose))
                if key == last_key:
                    pending.append(i)
                    total_removed += 1
                    continue
                last_key = key
                new.append(i)
            else:
                if "PE" in eng and t not in ("InstMatmult",
                                             "InstEventSemaphore"):
                    last_key = None  # unknown PE inst may clobber weights
                if pending and t == "InstMatmult":
                    for j in pending:
                        i.merge_dependencies_from(j)
                    pending = []
                new.append(i)
        if pending:
            new.extend(pending)
        blk.instructions = new
    return total_removed


def build_nc(reps: int = 1):
    nc = bacc.Bacc("TRN2", target_bir_lowering=False, debug=False,
                   num_devices=N_CORES)
    AF = mybir.ActivationFunctionType

    xt = nc.dram_tensor("xt", [KBX, BL], BF16, kind="ExternalInput").ap()
    ht = nc.dram_tensor("ht", [KBH, BL], BF16, kind="ExternalInput").ap()
    hb = nc.dram_tensor("hb", [BL, H], BF16, kind="ExternalInput").ap()
    wx = nc.dram_tensor("wx", [KBX, 3 * H], BF16, kind="ExternalInput").ap()
    wh = nc.dram_tensor("wh", [KBH, 3 * H], BF16, kind="ExternalInput").ap()
    bias = nc.dram_tensor("bias", [128, 3 * H], F32, kind="ExternalInput").ap()
    out = nc.dram_tensor("out", [BL, H], BF16, kind="ExternalOutput").ap()
    if KFX:
        xt8 = nc.dram_tensor("xt8", [KFX, BL], FP8, kind="ExternalInput").ap()
        wx8 = nc.dram_tensor("wx8", [KFX, 3 * H], FP8,
                             kind="ExternalInput").ap()
    if KFH:
        ht8 = nc.dram_tensor("ht8", [KFH, BL], FP8, kind="ExternalInput").ap()
        wh8 = nc.dram_tensor("wh8", [KFH, 3 * H], FP8,
                             kind="ExternalInput").ap()

    with tile.TileContext(nc) as tc, ExitStack() as ctx:
        wpool = ctx.enter_context(tc.tile_pool(name="w", bufs=1))
        apool = ctx.enter_context(tc.tile_pool(name="a", bufs=1))
        hpool = ctx.enter_context(tc.tile_pool(name="h", bufs=2))
        epool = ctx.enter_context(tc.tile_pool(name="e", bufs=3))
        psum = ctx.enter_context(tc.tile_pool(name="ps", bufs=1, space="PSUM"))

        wx_sb = wpool.tile([128, KOX, 3 * H], BF16, tag="wx")
        wh_sb = wpool.tile([128, KOH, 3 * H], BF16, tag="wh")
        bias_sb = wpool.tile([128, 3 * H], F32, tag="bias")
        xt_sb = apool.tile([128, KOX, BL], BF16, tag="xt")
        ht_sb = apool.tile([128, KOH, BL], BF16, tag="ht")
        if KFX:
            wx8_sb = wpool.tile([128, 2 * KPX, 3 * H], FP8, tag="wx8")
            xt8_sb = apool.tile([128, 2 * KPX, BL], FP8, tag="xt8")
        if KFH:
            wh8_sb = wpool.tile([128, 2 * KPH, 3 * H], FP8, tag="wh8")
            ht8_sb = apool.tile([128, 2 * KPH, BL], FP8, tag="ht8")

        xt_r = xt.rearrange("(ko ki) b -> ki ko b", ki=128)
        ht_r = ht.rearrange("(ko ki) b -> ki ko b", ki=128)
        wx_r = wx.rearrange("(ko ki) n -> ki ko n", ki=128)
        wh_r = wh.rearrange("(ko ki) n -> ki ko n", ki=128)
        if KFX:
            xt8_r = xt8.rearrange("(ko ki) b -> ki ko b", ki=128)
            wx8_r = wx8.rearrange("(ko ki) n -> ki ko n", ki=128)
        if KFH:
            ht8_r = ht8.rearrange("(ko ki) b -> ki ko b", ki=128)
            wh8_r = wh8.rearrange("(ko ki) n -> ki ko n", ki=128)

        def emit_loads():
            # Load order sets DMA FIFO order on the sync ring. m-tile 0 runs
            # gate-major (r, z, a) over 512-col weight halves, so stream in
            # exactly that consumption order; fine chunks let the next rep's
            # first m-tiles start while the rest still loads.
            CH = 4
            cw = BL // CH
            nc.sync.dma_start(xt_sb[:, :, 0:cw], xt_r[:, :, 0:cw])
            if KFX:
                nc.sync.dma_start(xt8_sb[:, :, 0:cw], xt8_r[:, :, 0:cw])
            for g in range(3):
                for nh in range(2):
                    c0 = g * H + nh * 512
                    nc.sync.dma_start(wx_sb[:, :, c0:c0 + 512],
                                      wx_r[:, :, c0:c0 + 512])
                    if KFX:
                        nc.sync.dma_start(wx8_sb[:, :, c0:c0 + 512],
                                          wx8_r[:, :, c0:c0 + 512])
            nc.sync.dma_start(ht_sb[:, :, 0:cw], ht_r[:, :, 0:cw])
            if KFH:
                nc.sync.dma_start(ht8_sb[:, :, 0:cw], ht8_r[:, :, 0:cw])
            for g in range(3):
                for nh in range(2):
                    c0 = g * H + nh * 512
                    nc.sync.dma_start(wh_sb[:, :, c0:c0 + 512],
                                      wh_r[:, :, c0:c0 + 512])
                    if KFH:
                        nc.sync.dma_start(wh8_sb[:, :, c0:c0 + 512],
                                          wh8_r[:, :, c0:c0 + 512])
            nc.sync.dma_start(bias_sb[:], bias)
            for c in range(1, CH):
                cs = slice(c * cw, (c + 1) * cw)
                nc.sync.dma_start(xt_sb[:, :, cs], xt_r[:, :, cs])
                nc.sync.dma_start(ht_sb[:, :, cs], ht_r[:, :, cs])
                if KFX:
                    nc.sync.dma_start(xt8_sb[:, :, cs], xt8_r[:, :, cs])
                if KFH:
                    nc.sync.dma_start(ht8_sb[:, :, cs], ht8_r[:, :, cs])

        def emit_mtile(mt):
            ms = slice(mt * 128, (mt + 1) * 128)
            hb_t = hpool.tile([128, H], BF16, tag="hb")
            nc.sync.dma_start(hb_t[:], hb[ms, :])

            # 8 PSUM banks: r/z accumulate x-side + h-side; 'a' is x@W_i2h
            # (+bias later), 'b' is h@W_h2h (scaled by r later). Each bank
            # takes KOX+KPX x-side ops and/or KOH+KPH h-side ops.
            ps = {}
            for g in ("r", "z", "a", "b"):
                for nh in range(2):
                    ps[(g, nh)] = psum.tile([128, 512], F32, tag=f"p{g}{nh}",
                                            name=f"p{g}{nh}")

            def mm_x(gi, g, ko, nh):
                o = gi * H + nh * 512
                nc.tensor.matmul(ps[(g, nh)], xt_sb[:, ko, ms],
                                 wx_sb[:, ko, o:o + 512],
                                 start=(ko == 0),
                                 stop=(g == "a" and KPX == 0
                                       and ko == KOX - 1))

            def mm_x8(gi, g, kp, nh):
                o = gi * H + nh * 512
                nc.tensor.matmul(ps[(g, nh)], xt8_sb[:, 2 * kp:2 * kp + 2, ms],
                                 wx8_sb[:, 2 * kp:2 * kp + 2, o:o + 512],
                                 start=False,
                                 stop=(g == "a" and kp == KPX - 1),
                                 perf_mode=DRMODE)

            def mm_h(gi, g, ko, nh):
                o = gi * H + nh * 512
                nc.tensor.matmul(ps[(g, nh)], ht_sb[:, ko, ms],
                                 wh_sb[:, ko, o:o + 512],
                                 start=(g == "b" and ko == 0),
                                 stop=(KPH == 0 and ko == KOH - 1))

            def mm_h8(gi, g, kp, nh):
                o = gi * H + nh * 512
                nc.tensor.matmul(ps[(g, nh)], ht8_sb[:, 2 * kp:2 * kp + 2, ms],
                                 wh8_sb[:, 2 * kp:2 * kp + 2, o:o + 512],
                                 start=False,
                                 stop=(kp == KPH - 1),
                                 perf_mode=DRMODE)

            def x_side_gate_major():
                for gi, g in enumerate(("r", "z", "a")):
                    for nh in range(2):
                        for ko in range(KOX):
                            mm_x(gi, g, ko, nh)
                        for kp in range(KPX):
                            mm_x8(gi, g, kp, nh)

            def x_side_ko_major():
                for ko in range(KOX):
                    for nh in range(2):
                        for gi, g in enumerate(("r", "z", "a")):
                            mm_x(gi, g, ko, nh)
                for kp in range(KPX):
                    for nh in range(2):
                        for gi, g in enumerate(("r", "z", "a")):
                            mm_x8(gi, g, kp, nh)

            def h_side_ko_major():
                for ko in range(KOH):
                    for nh in range(2):
                        for gi, g in enumerate(("r", "z", "b")):
                            mm_h(gi, g, ko, nh)
                for kp in range(KPH):
                    for nh in range(2):
                        for gi, g in enumerate(("r", "z", "b")):
                            mm_h8(gi, g, kp, nh)

            def h_side_gate_major(order=("r", "z", "b"), nhs=(0, 1)):
                h_idx = {"r": 0, "z": 1, "b": 2}
                for g in order:
                    for nh in nhs:
                        for ko in range(KOH):
                            mm_h(h_idx[g], g, ko, nh)
                        for kp in range(KPH):
                            mm_h8(h_idx[g], g, kp, nh)

            def epilogue(nh):
                o = nh * 512
                nsl = slice(o, o + 512)
                pr, pz = ps[("r", nh)], ps[("z", nh)]
                pa, pb = ps[("a", nh)], ps[("b", nh)]
                tr = epool.tile([128, 512], F32, tag="tr")
                tz = epool.tile([128, 512], F32, tag="tz")
                ta = epool.tile([128, 512], F32, tag="ta")
                nc.vector.tensor_add(tr[:], pr[:], bias_sb[:, o:o + 512])
                nc.scalar.activation(tr[:], tr[:], AF.Sigmoid)       # r
                nc.vector.tensor_add(tz[:], pz[:], bias_sb[:, H + o:H + o + 512])
                nc.scalar.activation(tz[:], tz[:], AF.Sigmoid)       # z
                nc.vector.tensor_add(ta[:], pa[:],
                                     bias_sb[:, 2 * H + o:2 * H + o + 512])
                nc.vector.tensor_mul(tr[:], tr[:], pb[:])            # r*(hU)
                nc.vector.tensor_add(ta[:], ta[:], tr[:])
                nc.scalar.activation(ta[:], ta[:], AF.Tanh)          # h1
                nc.vector.tensor_sub(tr[:], hb_t[:, nsl], ta[:])     # h-h1
                nc.vector.tensor_mul(tr[:], tz[:], tr[:])            # z*(h-h1)
                to = epool.tile([128, 512], BF16, tag="to")
                nc.vector.tensor_add(to[:], ta[:], tr[:])            # out
                nc.scalar.dma_start(out[ms, nsl], to[:])

            if mt == 0:
                # Gate-major: PE starts after ~1MB of weights instead of
                # all of them (extra LDWEIGHTS are free - fully hidden).
                x_side_gate_major()
                h_side_gate_major()
                epilogue(0)
                epilogue(1)
            elif mt < MT - 1:
                x_side_ko_major()
                h_side_ko_major()
                epilogue(0)
                epilogue(1)
            else:
                # Last m-tile: finish each nh half completely ('b' first,
                # so the epilogue's critical chain r*b -> tanh -> blend
                # starts sooner) and emit its epilogue before the other
                # half's matmuls - the nh=0 epilogue then overlaps the
                # nh=1 matmul stream instead of serializing after it.
                for nh in range(2):
                    for gi, g in enumerate(("r", "z", "a")):
                        for ko in range(KOX):
                            mm_x(gi, g, ko, nh)
                        for kp in range(KPX):
                            mm_x8(gi, g, kp, nh)
                    h_side_gate_major(order=("b", "r", "z"), nhs=(nh,))
                    epilogue(nh)

        def body():
            emit_loads()
            for mt in range(MT):
                emit_mtile(mt)

        if reps > 1:
            # tc.For_i has an all-engine barrier per iteration, which blocks
            # the next rep's input DMA from overlapping this rep's tail.
            # Unroll several bodies per iteration so tile-level region deps
            # pipeline rep boundaries for (unroll-1)/unroll of the reps.
            unroll = 1
            pref = int(os.environ.get("GRU_UNROLL", "8"))
            for u in (pref, 4, 3, 2):
                if u > 1 and reps % u == 0:
                    unroll = u
                    break
            with tc.For_i(0, reps // unroll, 1):
                for _ in range(unroll):
                    body()
        else:
            body()

    nc.compile()
    if os.environ.get("GRU_DEDUP", "1") == "1":
        dedupe_ldweights(nc)
    return nc


def prep_in_maps(inputs):
    """Host-side marshalling: shard batch, transpose+cast activations,
    concat weights/biases, split the fp8 contraction rows. Returns per-core
    input dicts."""
    g = {k: np.asarray(v) for k, v in inputs.items()}
    x, h = g["inputs"], g["hidden"]
    wx = np.ascontiguousarray(np.concatenate(
        [g["W_i2r"], g["W_i2z"], g["W_i2h"]], axis=1)).astype(np.float32)
    wh = np.ascontiguousarray(np.concatenate(
        [g["W_h2r"], g["W_h2z"], g["W_h2h"]], axis=1)).astype(np.float32)
    b = np.concatenate([g["b_i2r"], g["b_i2z"], g["b_i2h"]]).astype(np.float32)
    bias_b = np.ascontiguousarray(np.broadcast_to(b, (128, 3 * H)))
    xt_all = np.ascontiguousarray(x.T).astype(np.float32)
    ht_all = np.ascontiguousarray(h.T).astype(np.float32)

    shared = {
        "wx": wx[:KBX].astype(BF16_NP),
        "wh": wh[:KBH].astype(BF16_NP),
        "bias": bias_b,
    }
    if KFX:
        shared["wx8"] = (wx[KBX:] * S8).astype(FP8_NP)
    if KFH:
        shared["wh8"] = (wh[KBH:] * S8).astype(FP8_NP)

    in_maps = []
    for c in range(N_CORES):
        sl = slice(c * BL, (c + 1) * BL)
        m = {
            "xt": np.ascontiguousarray(xt_all[:KBX, sl]).astype(BF16_NP),
            "ht": np.ascontiguousarray(ht_all[:KBH, sl]).astype(BF16_NP),
            "hb": np.ascontiguousarray(h[sl]).astype(BF16_NP),
        }
        if KFX:
            m["xt8"] = np.ascontiguousarray(
                xt_all[KBX:, sl] / S8).astype(FP8_NP)
        if KFH:
            m["ht8"] = np.ascontiguousarray(
                ht_all[KBH:, sl] / S8).astype(FP8_NP)
        m.update(shared)
        in_maps.append(m)
    return in_maps


_RUNNERS = {}


def get_runner(reps: int = 1):
    """Build the bass module once and wrap it in a jitted 8-way shard_map,
    mirroring concourse.bass2jax.run_bass_via_pjrt but reusable across calls
    (so repeated executions don't re-trace/re-compile). reps>1 wraps the
    whole kernel in an on-device loop (for timing via amortization)."""
    if reps in _RUNNERS:
        return _RUNNERS[reps]
    import jax
    from jax.sharding import Mesh, PartitionSpec
    from jax.experimental.shard_map import shard_map
    from concourse.bass2jax import (_bass_exec_p, install_neuronx_cc_hook,
                                    partition_id_tensor)

    nc = build_nc(reps)
    install_neuronx_cc_hook()

    partition_name = (nc.partition_id_tensor.name
                      if nc.partition_id_tensor else None)
    in_names, out_names, out_avals, zero_outs = [], [], [], []
    for alloc in nc.m.functions[0].allocations:
        if not isinstance(alloc, mybir.MemoryLocationSet):
            continue
        name = alloc.memorylocations[0].name
        if alloc.kind == "ExternalInput":
            if name != partition_name:
                in_names.append(name)
        elif alloc.kind == "ExternalOutput":
            out_names.append(name)
            shape = tuple(alloc.tensor_shape)
            dtype = mybir.dt.np(alloc.dtype)
            out_avals.append(jax.core.ShapedArray(shape, dtype))
            zero_outs.append(np.zeros(shape, dtype))
    all_names = in_names + out_names
    if partition_name is not None:
        all_names = all_names + [partition_name]
    all_names = tuple(all_names)
    n_in, n_out = len(in_names), len(out_names)

    def _body(*args):
        operands = list(args)
        if partition_name is not None:
            operands.append(partition_id_tensor())
        outs = _bass_exec_p.bind(
            *operands,
            out_avals=tuple(out_avals),
            in_names=all_names,
            out_names=tuple(out_names),
            lowering_input_output_aliases=(),
            sim_require_finite=True,
            sim_require_nnan=True,
            nc=nc,
        )
        return tuple(outs)

    devices = jax.devices()[:N_CORES]
    mesh = Mesh(np.asarray(devices), ("core",))
    sharded = jax.jit(
        shard_map(_body, mesh=mesh,
                  in_specs=(PartitionSpec("core"),) * (n_in + n_out),
                  out_specs=(PartitionSpec("core"),) * n_out,
                  check_rep=False),
        donate_argnums=tuple(range(n_in, n_in + n_out)),
        keep_unused=True,
    )
    _RUNNERS[reps] = (sharded, in_names, out_names, zero_outs)
    return _RUNNERS[reps]


def run_on_device(in_maps):
    sharded, in_names, out_names, zero_outs = get_runner()
    concat_in = [np.concatenate([m[n] for m in in_maps], axis=0)
                 for n in in_names]
    concat_zero = [np.zeros((N_CORES * z.shape[0], *z.shape[1:]), z.dtype)
                   for z in zero_outs]
    outs = sharded(*concat_in, *concat_zero)
    return {n: np.asarray(o) for n, o in zip(out_names, outs)}


_NC = None


def kernel(**inputs):
    """Full-input entry point: shard, run on 8 NeuronCores, gather."""
    global _NC
    from concourse._compat import axon_active
    in_maps = prep_in_maps(inputs)
    if axon_active():
        # PJRT path with a process-cached jitted executable (repeat calls
        # skip re-trace/re-compile).
        return run_on_device(in_maps)["out"].astype(np.float32)
    from concourse.bass_utils import run_bass_kernel_spmd
    if _NC is None:
        _NC = build_nc(1)
    res = run_bass_kernel_spmd(_NC, in_maps, core_ids=list(range(N_CORES)))
    return np.concatenate([res.results[c]["out"] for c in range(N_CORES)],
                          axis=0).astype(np.float32)
